# revision 25
# baseline (speedup 1.0000x reference)
"""Multi-head causal attention (B=4, S=2048, D=1024, H=16, dk=dv=64) on 8
Trainium2 NeuronCores.

Sharding: batch (4) x head-group (2) -> 8 cores. Each core computes, for its
batch b and its 8 heads, the partial output (concat_g @ WO_g)^T in [D, S]
layout. Host sums the two head-group partials per batch, transposes, adds bO.

Per-core kernel (single SPMD Bass program, per-core data):
  Projections: Q^T/K^T = (X @ W)^T in fp8e4 DoubleRow (weights pre-scaled by
  QK_WSCALE, folded back inside the exp scale); Vbar[s, h, 0:65] = V | ones
  in bf16.  These are NOT separate phases: they are emitted as "filler"
  chunks interleaved into the attention pipeline, because the scalar-engine
  exp otherwise runs 1:1 with the tensor engine and every jitter stalls the
  PE.  Each chunk is one wide [128,1024] PSUM tile's worth of matmuls.

  Attention (software-pipelined, skew 2, pair-grouped, q-blocks ascending):
  per iteration (q-block 512, head pair, k-chunk 128):
    front(i): scoresT[k, q] for both heads into one wide PSUM tile (K=64
    row-quadrants adjacent; head1's live block shifted to col 512 so the
    live columns [o, 1024-o) are contiguous -> ONE exp instr), tri-mask of
    the diagonal-crossing cols on DVE.
    back(i-2): fv_p[0:65, q] += Vbar-chunk.T @ at_p  (row 64 = denom).
    Every tensor matmul carries a nosync edge to the previous one, pinning
    engine order (score/fv alternation would flush the PE tile config).
    hp end: denom rows -> reciprocal_approx_fast -> gpsimd broadcast ->
    fused scale+cast into CT (bf16).
  Out-projection: per 2-mt group, lagged into the following q-block's
  stream (more act-free filler); PSUM evacuated via scalar/vector casts to
  bf16 and DMA'd out.
"""

import sys

sys.path.insert(0, "/opt/trn_rl_repo")

import numpy as np
import ml_dtypes

import concourse.bass as bass
from concourse import bacc
import concourse.tile as tile
from concourse import mybir
from concourse.bass_utils import run_bass_kernel_spmd

F32 = mybir.dt.float32
BF16 = mybir.dt.bfloat16
FP8 = mybir.dt.float8e4
DR = mybir.MatmulPerfMode.DoubleRow
EXP = mybir.ActivationFunctionType.Exp
COPY = mybir.ActivationFunctionType.Copy
ADT = BF16   # dtype for Q/K/V/attn tiles (attention matmuls)

# The Q/K projections run in fp8e4 DoubleRow mode. WQ/WK are pre-scaled by
# QK_WSCALE on the host (lifts the tiny uniform(-1/32,1/32) weights out of
# the fp8 subnormal range); the exp() then folds 1/QK_WSCALE^2 into its
# scale: exp(scores/8) = exp(scaled_scores * 0.125 / QK_WSCALE^2).
QK_WSCALE = 16.0
EXP_SCALE = 0.125 / (QK_WSCALE * QK_WSCALE)

S = 2048          # sequence length
D = 1024          # model dim
HG = 8            # heads per core
DK = 64           # head dim
JG = HG * DK      # 512 = projected dim per core
CK = D // 128     # 8 bf16 contraction chunks (V projection)
C2 = D // 256     # 4 fp8 DoubleRow contraction chunks (Q/K projections)
NJT = JG // 128   # 4 j-tiles (head pairs)
NST = S // 128    # 16 s-tiles of 128
NQB = S // 512    # 4 q-blocks of 512

SKEW = 2          # fv trails scores by this many (qb,hp,kc) iterations
PLAG = 6          # out-projection trails the end of its q-block by this many
FILL_RATE = 3     # emit a projection filler chunk every this many pair-steps

_NC_CACHE = {}


def build_nc(salt=""):
    nc = bacc.Bacc("TRN2", target_bir_lowering=False)

    XT_d = nc.declare_dram_parameter("XT", [D, S], BF16, isOutput=False)
    X8_d = nc.declare_dram_parameter("X8", [C2, 128, 2, S], FP8,
                                     isOutput=False)
    WQ8_d = nc.declare_dram_parameter("WQ8", [C2, 128, 2, JG], FP8,
                                      isOutput=False)
    WK8_d = nc.declare_dram_parameter("WK8", [C2, 128, 2, JG], FP8,
                                      isOutput=False)
    WV_d = nc.declare_dram_parameter("WV", [D, JG], BF16, isOutput=False)
    WO_d = nc.declare_dram_parameter("WO", [JG, D], BF16, isOutput=False)
    OUT_d = nc.declare_dram_parameter("OUTT", [D, S], BF16, isOutput=True)

    # tril-inclusive mask in (k, q) orientation: mask[kr, qr] = 1 iff qr >= kr
    tri_np = (np.arange(128)[None, :] >= np.arange(128)[:, None]).astype(
        mybir.dt.np(ADT))
    TRI_d = nc.inline_tensor(tri_np, name="trimask" + salt)

    # Pin the tensor-engine order to emission order (see module docstring).
    import bass_rust as _br
    prev_mm = [None]

    def tmm(out, lhsT, rhs, **kw):
        mm = nc.tensor.matmul(out, lhsT, rhs, **kw)
        if prev_mm[0] is not None:
            deps = _br.InstructionNameOrderedSet()
            deps.add(prev_mm[0])
            mm.ins.add_nosync_dependencies_from(deps)
        prev_mm[0] = mm.ins.name
        return mm

    with tile.TileContext(nc) as tc:
        with (
            tc.tile_pool(name="persist", bufs=1) as pp,
            tc.tile_pool(name="psum", bufs=1, space="PSUM") as ps,
            tc.tile_pool(name="stage", bufs=1) as sp,
            tc.tile_pool(name="late", bufs=1) as lp,
        ):
            tri = pp.tile([128, 128], ADT, tag="tri", name="tri")
            nc.sync.dma_start(tri[:], TRI_d[:])

            QT = [pp.tile([128, S], ADT, tag=f"qt{j}", name=f"qt{j}")
                  for j in range(NJT)]
            KT = [pp.tile([128, S], ADT, tag=f"kt{j}", name=f"kt{j}")
                  for j in range(NJT)]
            VB = [pp.tile([128, HG, DK + 1], ADT, tag=f"vb{s}", name=f"vb{s}")
                  for s in range(NST)]

            X8 = [sp.tile([128, 2, S], FP8, tag=f"x8{c}", name=f"x8{c}")
                  for c in range(C2)]
            WQ8 = [sp.tile([128, 2, JG], FP8, tag=f"wq{c}", name=f"wq{c}")
                   for c in range(C2)]
            WK8 = [sp.tile([128, 2, JG], FP8, tag=f"wk{c}", name=f"wk{c}")
                   for c in range(C2)]
            XT = [sp.tile([128, S], BF16, tag=f"xt{c}", name=f"xt{c}")
                  for c in range(CK)]
            WV = [sp.tile([128, JG], BF16, tag=f"wv{c}", name=f"wv{c}")
                  for c in range(CK)]
            WO = [lp.tile([128, D], BF16, tag=f"wo{c}", name=f"wo{c}")
                  for c in range(NJT)]
            CT = [
                [lp.tile([128, 512], BF16, tag=f"ct{qb}_{hp}",
                         name=f"ct{qb}_{hp}") for hp in range(NJT)]
                for qb in range(NQB)
            ]

            # DMA emission in consumption order: fp8 Q/K inputs for the
            # preroll, then XT/WV column-quarter 0 (first V chunks), then
            # the rest.
            for c in range(C2):
                nc.sync.dma_start(X8[c][:, :, 0:1024], X8_d[c][:, :, 0:1024])
                nc.sync.dma_start(WQ8[c][:], WQ8_d[c])
            for c in range(C2):
                nc.sync.dma_start(WK8[c][:], WK8_d[c])
            for c in range(CK):
                nc.sync.dma_start(XT[c][:, 0:512], XT_d[128 * c : 128 * (c + 1),
                                                        0:512])
                nc.sync.dma_start(WV[c][:], WV_d[128 * c : 128 * (c + 1), :])
            for c in range(C2):
                nc.sync.dma_start(X8[c][:, :, 1024:2048],
                                  X8_d[c][:, :, 1024:2048])
            for c in range(NJT):
                nc.sync.dma_start(WO[c][:], WO_d[128 * c : 128 * (c + 1), :])
            for q in range(1, 4):
                for c in range(CK):
                    nc.sync.dma_start(
                        XT[c][:, 512 * q : 512 * (q + 1)],
                        XT_d[128 * c : 128 * (c + 1), 512 * q : 512 * (q + 1)])

            # ---- projection filler chunks ----
            def chunk_qk(W8, OT, jt, sbp):
                pq = ps.tile([128, 1024], F32, tag="scw", name="scw", bufs=2)
                for half in (0, 1):
                    for c in range(C2):
                        tmm(
                            pq[:, 512 * half : 512 * (half + 1)],
                            W8[c][:, :, 128 * jt : 128 * (jt + 1)],
                            X8[c][:, :, 1024 * sbp + 512 * half
                                  : 1024 * sbp + 512 * (half + 1)],
                            start=(c == 0),
                            stop=(c == C2 - 1),
                            perf_mode=DR,
                        )
                nc.vector.tensor_copy(
                    OT[jt][:, 1024 * sbp : 1024 * (sbp + 1)], pq[:]
                )

            def chunk_v(stp):
                pv = ps.tile([128, 1024], F32, tag="scw", name="scw", bufs=2)
                for half in (0, 1):
                    st = 2 * stp + half
                    for c in range(CK):
                        tmm(
                            pv[:, 512 * half : 512 * (half + 1)],
                            XT[c][:, 128 * st : 128 * (st + 1)],
                            WV[c][:],
                            start=(c == 0),
                            stop=(c == CK - 1),
                        )
                for half in (0, 1):
                    st = 2 * stp + half
                    nc.vector.tensor_copy(
                        VB[st][:, :, 0:DK],
                        pv[:, 512 * half : 512 * (half + 1)].rearrange(
                            "p (h d) -> p h d", h=HG),
                    )
                    nc.gpsimd.memset(VB[st][:, :, DK : DK + 1], 1.0)

            # filler queue: (deadline_iter, emit_fn).  Preroll (deadline 0)
            # runs before the pipeline; the rest are paced through it.
            qb_start = {}
            idx = 0
            for qb in range(NQB):
                qb_start[qb] = idx
                idx += NJT * (4 * qb + 4)
            fillers = []
            for hp in range(NJT):
                fillers.append((qb_start[0],
                                lambda hp=hp: chunk_qk(WQ8, QT, hp, 0)))
                fillers.append((qb_start[0],
                                lambda hp=hp: chunk_qk(WK8, KT, hp, 0)))
            fillers.append((qb_start[0], lambda: chunk_v(0)))
            fillers.append((qb_start[0], lambda: chunk_v(1)))
            fillers.append((qb_start[1], lambda: chunk_v(2)))
            fillers.append((qb_start[1], lambda: chunk_v(3)))
            for hp in range(NJT):
                fillers.append((qb_start[2],
                                lambda hp=hp: chunk_qk(WQ8, QT, hp, 1)))
                fillers.append((qb_start[2],
                                lambda hp=hp: chunk_qk(WK8, KT, hp, 1)))
            fillers.append((qb_start[2], lambda: chunk_v(4)))
            fillers.append((qb_start[2], lambda: chunk_v(5)))
            fillers.append((qb_start[3], lambda: chunk_v(6)))
            fillers.append((qb_start[3], lambda: chunk_v(7)))
            fillers.sort(key=lambda f: f[0])

            # ---- attention pipeline ----
            iters = [(qb, hp, kc)
                     for qb in range(NQB)
                     for hp in range(NJT)
                     for kc in range(4 * qb + 4)]
            n_it = len(iters)
            at_tiles = {}
            fv_tiles = {}
            pending_p4 = []
            done_hps = [0] * NQB

            def emit_front(i):
                qb, hp, kc = iters[i]
                o = max(0, 128 * kc - 512 * qb)
                # head0 live cols [o, 512), head1 live cols [512, 1024-o):
                # contiguous union -> single exp instr.
                sc = ps.tile([128, 1024], F32, tag="scw", name="scw", bufs=2)
                for p in (0, 1):
                    pr = 64 * p
                    lo = o if p == 0 else 512
                    tmm(
                        sc[:, lo : lo + 512 - o],
                        KT[hp][pr : pr + 64, 128 * kc : 128 * (kc + 1)],
                        QT[hp][pr : pr + 64, 512 * qb + o : 512 * (qb + 1)],
                        start=True,
                        stop=True,
                    )
                at = lp.tile([128, 1024], ADT, tag="at", name="at", bufs=4)
                nc.scalar.activation(at[:, o : 1024 - o], sc[:, o : 1024 - o],
                                     EXP, scale=EXP_SCALE)
                if kc >= 4 * qb:  # diagonal-crossing tile
                    for p in (0, 1):
                        lo = o if p == 0 else 512
                        blk = at[:, lo : lo + 128]
                        nc.vector.tensor_mul(blk, blk, tri[:])
                at_tiles[i] = at

            def emit_back(i):
                qb, hp, kc = iters[i]
                nkc = 4 * qb + 4
                o = max(0, 128 * kc - 512 * qb)
                if kc == 0:
                    fv_tiles[(qb, hp)] = [
                        ps.tile([DK + 1, 512], F32, tag=f"fv{p}",
                                name=f"fv{p}", bufs=2) for p in (0, 1)
                    ]
                fv = fv_tiles[(qb, hp)]
                at = at_tiles.pop(i)
                for p in (0, 1):
                    h = 2 * hp + p
                    lo = o if p == 0 else 512
                    tmm(
                        fv[p][:, o:512],
                        VB[kc][:, h, :],
                        at[:, lo : lo + 512 - o],
                        start=(kc == 0),
                        stop=(kc == nkc - 1),
                    )
                if kc == nkc - 1:
                    # head-pair epilogue: denominators -> recip ->
                    # broadcast -> fused scale+cast into CT
                    for p in (0, 1):
                        dr = lp.tile([1, 512], F32, tag=f"dr{p}",
                                     name=f"dr{p}", bufs=2)
                        nc.vector.tensor_copy(dr[:], fv[p][64:65, :])
                        nc.vector.reciprocal_approx_fast(dr[:], dr[:])
                        rb = lp.tile([128, 512], F32, tag=f"rb{p}",
                                     name=f"rb{p}", bufs=2)
                        nc.gpsimd.partition_broadcast(rb[:], dr[:])
                        nc.vector.tensor_mul(
                            CT[qb][hp][64 * p : 64 * (p + 1), :],
                            fv[p][0:64, :],
                            rb[64 * p : 64 * (p + 1), :],
                        )
                    done_hps[qb] += 1
                    if done_hps[qb] == NJT:
                        for mtp in range(D // 256):
                            pending_p4.append((i + PLAG + 2 * mtp, qb, mtp))

            def emit_p4_group(qb, mtp):
                po = ps.tile([128, 1024], F32, tag="scw", name="scw", bufs=2)
                for half in (0, 1):
                    mt = 2 * mtp + half
                    for c in range(NJT):
                        tmm(
                            po[:, 512 * half : 512 * (half + 1)],
                            WO[c][:, 128 * mt : 128 * (mt + 1)],
                            CT[qb][c][:],
                            start=(c == 0),
                            stop=(c == NJT - 1),
                        )
                for half in (0, 1):
                    mt = 2 * mtp + half
                    og = lp.tile([128, 512], BF16, tag="ostg", name="ostg",
                                 bufs=6)
                    osrc = po[:, 512 * half : 512 * (half + 1)]
                    # split the PSUM-evacuation casts across scalar and
                    # vector; the tail q-block goes all-scalar (exp done).
                    if qb == NQB - 1 or half == 1:
                        nc.scalar.activation(og[:], osrc, COPY)
                    else:
                        nc.vector.tensor_copy(og[:], osrc)
                    nc.sync.dma_start(
                        OUT_d[128 * mt : 128 * (mt + 1),
                              512 * qb : 512 * (qb + 1)],
                        og[:],
                    )

            fill_i = 0
            last_fill_step = -FILL_RATE

            def flush_fillers(deadline, step, rate_ok):
                nonlocal fill_i, last_fill_step
                while fill_i < len(fillers) and fillers[fill_i][0] <= deadline:
                    fillers[fill_i][1]()
                    fill_i += 1
                    last_fill_step = step
                if (rate_ok and fill_i < len(fillers)
                        and step - last_fill_step >= FILL_RATE):
                    fillers[fill_i][1]()
                    fill_i += 1
                    last_fill_step = step

            for g in range(0, n_it + SKEW, 2):
                for j in (g, g + 1):
                    if j < n_it:
                        flush_fillers(j, g, rate_ok=(j == g))
                        emit_front(j)
                for j in (g - SKEW, g + 1 - SKEW):
                    if 0 <= j < n_it:
                        emit_back(j)
                while pending_p4 and pending_p4[0][0] <= g + 1 - SKEW:
                    _, pqb, pmtp = pending_p4.pop(0)
                    emit_p4_group(pqb, pmtp)
            while fill_i < len(fillers):
                fillers[fill_i][1]()
                fill_i += 1
            while pending_p4:
                _, pqb, pmtp = pending_p4.pop(0)
                emit_p4_group(pqb, pmtp)
    nc.finalize()
    return nc


def _get_nc():
    if "nc" not in _NC_CACHE:
        _NC_CACHE["nc"] = build_nc()
    return _NC_CACHE["nc"]


def _dr_pack(arr, ncols):
    """[1024, ncols] -> [4, 128, 2, ncols] fp8e4 DoubleRow layout."""
    f8 = ml_dtypes.float8_e4m3
    return np.ascontiguousarray(
        arr.reshape(4, 2, 128, ncols).transpose(0, 2, 1, 3)).astype(f8)


def _make_in_maps(XKV, WQ, WK, WV, WO):
    bf = ml_dtypes.bfloat16
    in_maps = []
    xt_b = [np.ascontiguousarray(XKV[b].T) for b in range(4)]
    x8_b = [_dr_pack(x, S) for x in xt_b]
    for core in range(8):
        b, g = core // 2, core % 2
        sl = slice(512 * g, 512 * (g + 1))
        in_maps.append(
            {
                "XT": xt_b[b].astype(bf),
                "X8": x8_b[b],
                "WQ8": _dr_pack(QK_WSCALE * np.asarray(WQ[:, sl],
                                                       dtype=np.float32), JG),
                "WK8": _dr_pack(QK_WSCALE * np.asarray(WK[:, sl],
                                                       dtype=np.float32), JG),
                "WV": np.ascontiguousarray(WV[:, sl]).astype(bf),
                "WO": np.ascontiguousarray(WO[sl, :]).astype(bf),
            }
        )
    return in_maps


def _combine(results, bO):
    out = np.empty((4, S, D), dtype=np.float32)
    for b in range(4):
        acc = (results[2 * b]["OUTT"].astype(np.float32)
               + results[2 * b + 1]["OUTT"].astype(np.float32))
        out[b] = acc.T + bO[None, :]
    return out


def kernel(XKV, WQ, WK, WV, WO, bO):
    XKV = np.asarray(XKV, dtype=np.float32)
    nc = _get_nc()
    in_maps = _make_in_maps(XKV, np.asarray(WQ), np.asarray(WK), np.asarray(WV),
                            np.asarray(WO))
    res = run_bass_kernel_spmd(nc, in_maps, list(range(8)))
    return _combine(res.results, np.asarray(bO, dtype=np.float32))
